# revision 2
# baseline (speedup 1.0000x reference)
"""Trainium2 Bass kernel for nn_Decoder_26585847562664.

16-head causal attention decoder: B=2, S=2048, D=1024, HD=64.
Sharded over 8 NeuronCores as (batch x head-group): core c handles batch
c//4 and heads [4*(c%4), 4*(c%4)+4) -- Wq/Wk/Wv are split column-wise by
head group on the host, so every core's work is fully local (no
collectives).

Self-contained: hardcodes shapes, imports only the system concourse
stack.
"""

import json
import os
import sys
import types

import numpy as np
import ml_dtypes

B, S, D, H = 2, 2048, 1024, 16
HD = 64
NH = 4            # heads per core
OC = NH * HD      # 256 projection columns per core
NB = S // 128     # 16 row blocks
QC = S // 512     # 4 q-chunks of 512
DCH = D // 128    # 8 contraction chunks
SCALE = 1.0 / 32.0  # 1/sqrt(D)

BF16 = ml_dtypes.bfloat16

_cache = {}


# --------------------------------------------------------------------------
# environment shims (walrus single-wait limit, missing NTFF hook, no egress)
# --------------------------------------------------------------------------

def _install_shims():
    import concourse.bass as bass

    if not getattr(bass.Bass.to_json_bytes, "_wait_split", False):
        orig = bass.Bass.to_json_bytes

        def to_json_bytes(self):
            m = json.loads(orig(self))
            for fn in m.get("functions", []):
                for bb in fn.get("blocks", []):
                    out = []
                    for inst in bb.get("instructions", []):
                        si = inst.get("sync_info")
                        waits = (si or {}).get("on_wait") or []
                        if len(waits) > 1:
                            for j, w in enumerate(waits[:-1]):
                                out.append({
                                    "debug": inst.get("debug", 0),
                                    "engine": inst["engine"],
                                    "ins": [],
                                    "name": f"{inst['name']}ws{j}",
                                    "opcode": "NoOp",
                                    "outs": [],
                                    "sync_info": {"on_update": [], "on_wait": [w]},
                                })
                            si["on_wait"] = [waits[-1]]
                        out.append(inst)
                    bb["instructions"] = out
            return json.dumps(m).encode()

        to_json_bytes._wait_split = True
        bass.Bass.to_json_bytes = to_json_bytes

    try:
        import antenv
        try:
            from antenv import axon_hooks  # noqa: F401
        except ImportError:
            from trn_agent_boot.trn_boot import _ntff_profile_via_ctypes

            mod = types.ModuleType("antenv.axon_hooks")
            hook = [_ntff_profile_via_ctypes("/opt/axon/libaxon_pjrt.so")]
            mod.get_axon_ntff_profile_hook = lambda: hook[0]
            mod.set_axon_ntff_profile_hook = lambda h: hook.__setitem__(0, h)
            sys.modules["antenv.axon_hooks"] = mod
            antenv.axon_hooks = mod
    except Exception:
        pass

    try:
        from concourse import bass_utils
        bass_utils.upload_artifacts = lambda tmpdir: "local://skipped"
    except Exception:
        pass


# --------------------------------------------------------------------------
# mask block classification (host side)
# --------------------------------------------------------------------------

def _classify_mask(m2):
    """m2: [S, S] int array, m2[q, k] == 1 -> position attended.

    Returns (kind, mtile_idx, mtiles):
      kind[kb][qb]  in {0 zero, 1 full, 2 mixed}  (kb = kv block, qb = q block)
      mtile_idx[kb][qb] -> index into mtiles for mixed blocks
      mtiles: [U, 128, 128] bf16, already transposed to [kv_local, q_local]
    """
    kind = [[0] * NB for _ in range(NB)]
    idx = [[-1] * NB for _ in range(NB)]
    uniq = {}
    tiles = []
    for kb in range(NB):
        for qb in range(NB):
            blk = m2[qb * 128:(qb + 1) * 128, kb * 128:(kb + 1) * 128]
            s = int(blk.sum())
            if s == 0:
                kind[kb][qb] = 0
            elif s == 128 * 128:
                kind[kb][qb] = 1
            else:
                kind[kb][qb] = 2
                tT = np.ascontiguousarray(blk.T.astype(BF16))
                key = tT.tobytes()
                if key not in uniq:
                    uniq[key] = len(tiles)
                    tiles.append(tT)
                idx[kb][qb] = uniq[key]
    if len(tiles) > 32:
        raise ValueError(f"mask has {len(tiles)} unique mixed 128x128 blocks; "
                         "kernel supports <= 32")
    if tiles:
        mt = np.stack(tiles)
    else:
        mt = np.zeros((1, 128, 128), BF16)
    return kind, idx, mt


# --------------------------------------------------------------------------
# bass kernel builder
# --------------------------------------------------------------------------

def _build_nc(kind, mtile_idx, n_mtiles):
    import concourse.bass as bass
    import concourse.mybir as mybir
    import concourse.tile as tile

    f32 = mybir.dt.float32
    bf16 = mybir.dt.bfloat16
    AF = mybir.ActivationFunctionType

    nc = bass.Bass()
    xq = nc.declare_dram_parameter("xq", [S, D], bf16, isOutput=False)
    xk = nc.declare_dram_parameter("xk", [S, D], bf16, isOutput=False)
    xv = nc.declare_dram_parameter("xv", [S, D], bf16, isOutput=False)
    wqT = nc.declare_dram_parameter("wqT", [D, OC], bf16, isOutput=False)
    wkT = nc.declare_dram_parameter("wkT", [D, OC], bf16, isOutput=False)
    wvT = nc.declare_dram_parameter("wvT", [D, OC], bf16, isOutput=False)
    bq2 = nc.declare_dram_parameter("bq2", [128, 2], f32, isOutput=False)
    bk2 = nc.declare_dram_parameter("bk2", [128, 2], f32, isOutput=False)
    bv1 = nc.declare_dram_parameter("bv1", [1, OC], bf16, isOutput=False)
    mtd = nc.declare_dram_parameter("mtiles", [n_mtiles, 128, 128], bf16,
                                    isOutput=False)
    out = nc.declare_dram_parameter("out", [S, OC], f32, isOutput=True)

    # last unmasked kv block per q block (for matmul stop flags)
    last_kb = [max((kb for kb in range(NB) if kind[kb][qb]), default=-1)
               for qb in range(NB)]
    # kv blocks needed per q chunk
    kbs_for_qc = [
        [kb for kb in range(NB)
         if any(kind[kb][4 * qc + j] for j in range(4))]
        for qc in range(QC)
    ]

    with tile.TileContext(nc) as tc:
        with (
            tc.tile_pool(name="consts", bufs=1) as cp,
            tc.tile_pool(name="weights", bufs=1) as wp,
            tc.tile_pool(name="persist", bufs=1) as pp,
            tc.tile_pool(name="xt", bufs=16) as xtp,
            tc.tile_pool(name="ptile", bufs=3) as ptp,
            tc.tile_pool(name="stage", bufs=8) as stp,
        ):
            # ---- constants / small loads ----
            w_sb = {}
            for name, dram in (("q", wqT), ("k", wkT), ("v", wvT)):
                t = wp.tile([128, DCH, OC], bf16, tag=f"w{name}")
                nc.gpsimd.dma_start(
                    out=t, in_=dram[:].rearrange("(dc p) o -> p dc o", p=128))
                w_sb[name] = t
            bq_sb = cp.tile([128, 2], f32, tag="bq")
            nc.gpsimd.dma_start(out=bq_sb, in_=bq2[:])
            bk_sb = cp.tile([128, 2], f32, tag="bk")
            nc.gpsimd.dma_start(out=bk_sb, in_=bk2[:])
            bv_sb = cp.tile([1, OC], bf16, tag="bv")
            nc.gpsimd.dma_start(out=bv_sb, in_=bv1[:])
            ones1 = cp.tile([1, 128], bf16, tag="ones")
            nc.vector.memset(ones1, 1.0)
            mt_sb = cp.tile([128, n_mtiles, 128], bf16, tag="mt")
            nc.gpsimd.dma_start(
                out=mt_sb, in_=mtd[:].rearrange("u p f -> p u f"))

            # persistent projected tensors
            qT_sb = pp.tile([128, 2, S], bf16, tag="qT")   # [o_local, og, s]
            kT_sb = pp.tile([128, 2, S], bf16, tag="kT")
            v_sb = pp.tile([128, NB * NH, HD + 1], bf16, tag="v")
            nc.vector.memset(v_sb[:, :, HD:HD + 1], 1.0)

            # ---- projections ----
            with tc.tile_pool(name="pjps", bufs=4, space="PSUM") as pjp:
                for name, xdram, dst, bias in (
                    ("q", xq, qT_sb, bq_sb),
                    ("k", xk, kT_sb, bk_sb),
                ):
                    with nc.named_scope(f"proj_{name}"):
                        xts = []
                        for d in range(DCH):
                            xt = xtp.tile([128, S], bf16, tag="xt")
                            nc.sync.dma_start(
                                out=xt, in_=xdram[:, d * 128:(d + 1) * 128],
                                transpose=True)
                            xts.append(xt)
                        for og in range(2):
                            for sc in range(QC):
                                ps = pjp.tile([128, 512], f32, tag="pjps")
                                for d in range(DCH):
                                    nc.tensor.matmul(
                                        ps,
                                        w_sb[name][:, d, og * 128:(og + 1) * 128],
                                        xts[d][:, sc * 512:(sc + 1) * 512],
                                        start=(d == 0), stop=(d == DCH - 1))
                                nc.scalar.activation(
                                    out=dst[:, og, sc * 512:(sc + 1) * 512],
                                    in_=ps, func=AF.Identity,
                                    bias=bias[:, og:og + 1], scale=1.0)
                with nc.named_scope("proj_v"):
                    xts = []
                    for d in range(DCH):
                        xt = xtp.tile([128, S], bf16, tag="xt")
                        nc.sync.dma_start(
                            out=xt, in_=xv[:, d * 128:(d + 1) * 128],
                            transpose=True)
                        xts.append(xt)
                    for sb in range(NB):
                        ps = pjp.tile([128, OC], f32, tag="vps")
                        for d in range(DCH):
                            nc.tensor.matmul(
                                ps, xts[d][:, sb * 128:(sb + 1) * 128],
                                w_sb["v"][:, d, :],
                                start=(d == 0), stop=False)
                        nc.tensor.matmul(ps, ones1, bv_sb,
                                         start=False, stop=True)
                        for h in range(NH):
                            nc.vector.tensor_copy(
                                v_sb[:, sb * NH + h, 0:HD],
                                ps[:, h * HD:(h + 1) * HD])

            # ---- attention ----
            with (
                tc.tile_pool(name="stps", bufs=2, space="PSUM") as sp,
                tc.tile_pool(name="ops", bufs=4, space="PSUM") as op,
            ):
                for h in range(NH):
                    og, ph = divmod(h, 2)
                    for qc in range(QC):
                        with nc.named_scope(f"attn_h{h}_qc{qc}"):
                            kbs = kbs_for_qc[qc]
                            o_ps = [op.tile([128, HD + 1], f32, tag="ops",
                                            name=f"o_ps{j}")
                                    for j in range(4)]
                            started = [False] * 4
                            for p0 in range(0, len(kbs), 2):
                                pair = kbs[p0:p0 + 2]
                                w = len(pair) * 512
                                st = sp.tile([128, 1024], f32, tag="stps")
                                for i, kb in enumerate(pair):
                                    nc.tensor.matmul(
                                        st[:, i * 512:(i + 1) * 512],
                                        kT_sb[ph * 64:(ph + 1) * 64, og,
                                              kb * 128:(kb + 1) * 128],
                                        qT_sb[ph * 64:(ph + 1) * 64, og,
                                              qc * 512:(qc + 1) * 512],
                                        start=True, stop=True)
                                pt = ptp.tile([128, 1024], bf16, tag="pt")
                                nc.scalar.activation(
                                    out=pt[:, 0:w], in_=st[:, 0:w],
                                    func=AF.Exp, scale=SCALE)
                                for i, kb in enumerate(pair):
                                    for j in range(4):
                                        qb = 4 * qc + j
                                        bk = kind[kb][qb]
                                        if bk == 0:
                                            continue
                                        sl = pt[:, i * 512 + j * 128:
                                                i * 512 + (j + 1) * 128]
                                        if bk == 2:
                                            u = mtile_idx[kb][qb]
                                            nc.vector.tensor_mul(
                                                sl, sl, mt_sb[:, u, :])
                                        nc.tensor.matmul(
                                            o_ps[j],
                                            sl,
                                            v_sb[:, kb * NH + h, :],
                                            start=(not started[j]),
                                            stop=(kb == last_kb[qb]))
                                        started[j] = True
                            for j in range(4):
                                qb = 4 * qc + j
                                if not started[j]:
                                    zb = stp.tile([128, HD], f32, tag="ob")
                                    nc.vector.memset(zb, 0.0)
                                    nc.gpsimd.dma_start(
                                        out=out[qb * 128:(qb + 1) * 128,
                                                h * HD:(h + 1) * HD],
                                        in_=zb)
                                    continue
                                rec = stp.tile([128, 1], f32, tag="rec")
                                nc.vector.reciprocal(
                                    rec, o_ps[j][:, HD:HD + 1])
                                ob = stp.tile([128, HD], f32, tag="ob")
                                nc.vector.tensor_scalar_mul(
                                    ob, o_ps[j][:, 0:HD], rec)
                                nc.gpsimd.dma_start(
                                    out=out[qb * 128:(qb + 1) * 128,
                                            h * HD:(h + 1) * HD],
                                    in_=ob)
    return nc


# --------------------------------------------------------------------------
# entry point
# --------------------------------------------------------------------------

def kernel(qx, kx, vx, mask, Wq, bq, Wk, bk, Wv, bv):
    _install_shims()
    from concourse.bass_utils import run_bass_kernel_spmd

    qx = np.asarray(qx)
    kx = np.asarray(kx)
    vx = np.asarray(vx)
    mask = np.asarray(mask)
    Wq = np.asarray(Wq, np.float32)
    bq = np.asarray(bq, np.float32)
    Wk = np.asarray(Wk, np.float32)
    bk = np.asarray(bk, np.float32)
    Wv = np.asarray(Wv, np.float32)
    bv = np.asarray(bv, np.float32)

    m2 = mask.reshape(S, S)
    kind, mtile_idx, mtiles = _classify_mask(m2)

    key = (tuple(tuple(r) for r in kind),
           tuple(tuple(r) for r in mtile_idx), mtiles.shape[0])
    if key not in _cache:
        _cache[key] = _build_nc(kind, mtile_idx, mtiles.shape[0])
    nc = _cache[key]

    in_maps = []
    for c in range(8):
        b, hg = divmod(c, 4)
        sl = slice(hg * OC, (hg + 1) * OC)
        in_maps.append({
            "xq": np.ascontiguousarray(qx[b].astype(BF16)),
            "xk": np.ascontiguousarray(kx[b].astype(BF16)),
            "xv": np.ascontiguousarray(vx[b].astype(BF16)),
            "wqT": np.ascontiguousarray(Wq[sl].T.astype(BF16)),
            "wkT": np.ascontiguousarray(Wk[sl].T.astype(BF16)),
            "wvT": np.ascontiguousarray(Wv[sl].T.astype(BF16)),
            "bq2": np.ascontiguousarray(bq[sl].reshape(2, 128).T,
                                        dtype=np.float32),
            "bk2": np.ascontiguousarray(bk[sl].reshape(2, 128).T,
                                        dtype=np.float32),
            "bv1": np.ascontiguousarray(bv[sl].reshape(1, OC).astype(BF16)),
            "mtiles": mtiles,
        })

    trace = os.environ.get("BASS_KERNEL_TRACE") == "1"
    if trace:
        # warm run first: profiling start before the first executable load
        # wedges the load under axon
        run_bass_kernel_spmd(nc, in_maps, list(range(8)), trace=False)
    res = run_bass_kernel_spmd(nc, in_maps, list(range(8)), trace=trace)
    if trace:
        print(f"HW exec time: {res.exec_time_ns} ns "
              f"(mean {res.mean_exec_time_ns})")

    outp = np.zeros((B, S, D), np.float32)
    for c in range(8):
        b, hg = divmod(c, 4)
        outp[b, :, hg * OC:(hg + 1) * OC] = res.results[c]["out"]
    return outp
